# revision 34
# baseline (speedup 1.0000x reference)
"""Trainium2 Bass kernel for nn_BinaryLinear (binarized 4-layer MLP + BatchNorm).

Reference computation (fp32, jax):
    h = x.reshape(-1, 3072)
    h = relu(h @ sign(W1).T); h = BN(h, g1, b1)   # BN over full 8192 batch
    h = relu(h @ sign(W2).T); h = BN(h, g2, b2)
    h = relu(h @ sign(W3).T); h = BN(h, g3, b3)
    out = h @ sign(W4).T                          # [8192, 10]

Strategy (8 NeuronCores, data-parallel over batch; ~237-276us on silicon —
the spread is the ncfw collective-firmware wake, 60-105us run-to-run; the
first two stats meshes complete at wake_end+~40 and wake_end+~63 regardless
of when they start, and everything downstream is ~103us of stable work):
  - Host: binarize weights to fp8/bf16 (+-1 exact), pack partition-major
    ([128, ktiles, free]) so DMAs are fat 2D-contiguous transfers, shard x
    over cores (1024 rows each), pre-scale BN gammas by sqrt(B).
  - Device (SPMD identical program): activations live feature-major
    [feature_part, batch_free] in SBUF. Each layer is a K-tiled bf16
    matmul accumulating in PSUM; the first 4 feature tiles run k-outer
    (layer-1 consumes input tiles in DMA arrival order), the rest as
    sequential chains. Matmul cadence is ~263ns per 512-col MM (silicon
    floor; hiding LDWEIGHTS does not change it). Relu on ScalarE (batch
    sum free via accum_out); sum(relu^2) on VectorE (stt h*h).
  - BatchNorm over the full 8192 batch via one AllGather per feature
    group + local tree-sum (payload [128, 2n] f32; transposed gathers
    lower to element-strided DMA ~17us, keep [part, core, stat]). The CC
    engine is serial and meshes are slow until ncfw init settles, so
    groups are [6,2]/[4,4]/[4,4]: L1's 6-group exports as early as
    possible (its mesh ends ~wake+40, delivering 6/8 k-tiles for L2's
    first phase), later layers use warm ~6us meshes. Stats chain is
    var-free: V = E2 - E1^2/B, a = g*sqrt(B)*sqrt(1/V) (one reciprocal +
    one Sqrt; Rsqrt activation is blocked for accuracy). Queue
    discipline: exports + triggers on GpSimd, gather-ins on Sync, stats
    math + applies on Vector, Sqrt on Scalar. A warmup AllGather at
    kernel start absorbs the ncfw wake off the critical path; act tables
    (Relu/Sqrt/Identity) are warmed at t~10us so no mid-kernel load.
  - Layer 4 folds BN3 into the weights (W4' trick): out = sum_k
    (W4[k]*a_k) @ relu_raw[k] + bias, the bias accumulated via a 1-column
    matmul of c/a sharing the scaled stationary weights; L4's matmuls
    chase each stats group directly with no H-apply chain on the tail,
    and the PSUM->SBUF drain fuses the +bias on both engine halves.
  - Post-compile, _dedup_ldweights removes the 326 redundant second-half
    weight loads from the BIR (halves Tensor-queue pressure).
  - Feed: XT/W2 on the Sync HWDGE ring, W1/W3/W4 on the Scalar ring, in
    growing chunks so the first accumulation chains start early.
"""
import os
import sys

for _p in ("/opt/trn_rl_repo",):
    if os.path.isdir(_p) and _p not in sys.path:
        sys.path.insert(0, _p)

import numpy as np
import ml_dtypes

from concourse import bacc, tile, mybir
from concourse import bass_utils

NCORES = 8
B = 8192
BL = B // NCORES            # 1024 rows per core
KIN = 3072
KT_IN = KIN // 128          # 24 k-tiles for layer 1
HID = 1024
JT = HID // 128             # 8 feature tiles
CLS = 10
CLSP = 16                   # padded classes
EPS = 1e-5
BF16 = mybir.dt.bfloat16
FP8 = mybir.dt.float8e4   # +-1 weights are exact in fp8; halves weight DMA traffic
F32 = mybir.dt.float32
ADD = mybir.AluOpType.add
SUB = mybir.AluOpType.subtract
MUL = mybir.AluOpType.mult
RELU = mybir.ActivationFunctionType.Relu
SQRT = mybir.ActivationFunctionType.Sqrt
MAX = mybir.AluOpType.max

_CACHE = {}


def _dedup_ldweights(nc):
    """Drop redundant InstLdweights from the compiled BIR.

    The rust lowering splits every matmul into InstLdweights +
    InstMatmult(ldweights=False), one load per matmul, even when
    consecutive matmuls use the identical stationary tile (the two
    512-col halves of each K-tile pair). The exposed load costs ~50ns
    per matmul on the PE array (measured 263ns/MM vs 213ns streaming).
    Here we remove an InstLdweights when (a) the previous InstLdweights
    in the same block loaded the exact same weights (same lowered AP,
    perf_mode, tile_position, transpose), (b) no other array-state
    change sits between them (only InstLdweights writes the array;
    matmuls/semaphores don't), and (c) it carries no semaphore waits or
    updates (so dropping it cannot break synchronization -- the waits
    that matter were moved onto the FIRST load of each pair by
    move_matmul_waits_to_ldweights).
    """
    removed = kept = 0
    for b in nc.m.functions[0].blocks:
        insts = b.instructions
        drop = []
        prev_key = None
        for i, ins in enumerate(insts):
            if isinstance(ins, mybir.InstLdweights):
                key = (
                    str(ins.ins[0]),
                    str(ins.perf_mode),
                    str(ins.tile_position),
                    str(ins.is_transpose),
                )
                si = ins.sync_info
                clean = si is None or (
                    len(si.on_wait) == 0 and len(si.on_update) == 0
                )
                if key == prev_key and clean:
                    drop.append(i)
                    removed += 1
                else:
                    kept += 1
                prev_key = key
        for i in reversed(drop):
            del insts[i]
    return removed, kept


def _build():
    nc = bacc.Bacc("TRN2", target_bir_lowering=False, debug=False, num_devices=NCORES)

    # All bulk inputs are partition-major on the host ([128, ktiles, free])
    # so DMAs are cheap-descriptor 2D patterns at full bandwidth.
    xt_d = nc.dram_tensor("xt", [128, KT_IN, BL], BF16, kind="ExternalInput")
    w1_d = nc.dram_tensor("w1t", [128, KT_IN, HID], FP8, kind="ExternalInput")
    w2_d = nc.dram_tensor("w2t", [128, JT, HID], FP8, kind="ExternalInput")
    w3_d = nc.dram_tensor("w3t", [128, JT, HID], FP8, kind="ExternalInput")
    w4_d = nc.dram_tensor("w4t", [128, JT, CLSP], BF16, kind="ExternalInput")
    bnp_d = nc.dram_tensor("bnp", [128, 6 * JT], F32, kind="ExternalInput")
    out_d = nc.dram_tensor("out", [CLSP, BL], F32, kind="ExternalOutput")

    nhalves = [(s, min(512, BL - s)) for s in range(0, BL, 512)]

    with tile.TileContext(nc) as tc:
        with (
            tc.tile_pool(name="weights", bufs=1) as wpool,
            tc.tile_pool(name="acts", bufs=1) as apool,
            tc.tile_pool(name="scratch", bufs=2) as scrpool,
            tc.tile_pool(name="stats", bufs=2) as spool,
            tc.tile_pool(name="psum", bufs=4, space="PSUM") as pspool,
            tc.tile_pool(name="dram", bufs=2, space="DRAM") as dpool,
        ):
            XT = wpool.tile([128, KT_IN, BL], BF16, tag="XT")
            W1 = wpool.tile([128, KT_IN, HID], FP8, tag="W1")
            W2 = wpool.tile([128, JT, HID], FP8, tag="W2")
            W3 = wpool.tile([128, JT, HID], FP8, tag="W3")
            W4 = wpool.tile([128, JT, CLSP], BF16, tag="W4")
            BNP = wpool.tile([128, 6 * JT], F32, tag="BNP")
            HRAW = apool.tile([128, JT, BL], BF16, tag="HRAW")
            H = apool.tile([128, JT, BL], BF16, tag="H")

            # One warmup collective: absorbs the ncfw wake + init cost
            # (~60-110us, run-variable) off the critical path. Input is an
            # unwritten scratch buffer (contents irrelevant); output
            # anchored into an unused out_d row at program end so DCE
            # keeps it. Collective latency stays elevated (~12-40us/mesh,
            # environmental) until ncfw's background init finishes around
            # 150us — extra or shape-matched warmups were measured NOT to
            # help, so keep exactly one and minimize early-era collectives.
            wu_outs = []
            for wi, cols in enumerate((1,)):
                win = dpool.tile([128, cols], F32, tag=f"wu_in{wi}")
                wo = dpool.tile([NCORES * 128, cols], F32, tag=f"wu_out{wi}")
                nc.gpsimd.collective_compute(
                    "AllGather",
                    mybir.AluOpType.bypass,
                    replica_groups=[list(range(NCORES))],
                    ins=[win.opt()],
                    outs=[wo.opt()],
                )
                wu_outs.append(wo)

            # Warm the Scalar activation tables (Relu + Rsqrt) at program
            # start so no ACT_TABLE_LOAD (~1.3us) lands mid-kernel on the
            # stats critical path. Input is uninitialized scratch (values
            # irrelevant, output discarded).
            WARM = scrpool.tile([128, 1], F32, tag="warm")
            WARMO = scrpool.tile([128, 1], F32, tag="warmo")
            nc.vector.memset(WARM[:], 1.0)
            nc.scalar.activation(WARMO[:], WARM[:], RELU)
            nc.scalar.activation(WARMO[:], WARM[:], SQRT)
            nc.scalar.activation(
                WARMO[:], WARM[:], mybir.ActivationFunctionType.Identity
            )

            # Input feed: XT on the Sync HWDGE ring, W1 on the Scalar HWDGE
            # ring, in progressively larger chunks so the first accumulation
            # chains start early while the bulk still moves in fat transfers.
            nc.sync.dma_start(BNP[:], bnp_d[:])
            # Tiny first transfers so the very first matmul (k=0, j=0,
            # batch half 0) can start ~1.5us earlier, then the bulk feed.
            nc.sync.dma_start(XT[:, 0:1, 0:512], xt_d[:, 0:1, 0:512])
            nc.scalar.dma_start(W1[:, 0:1, 0:128], w1_d[:, 0:1, 0:128])
            nc.sync.dma_start(XT[:, 0:1, 512:BL], xt_d[:, 0:1, 512:BL])
            nc.scalar.dma_start(W1[:, 0:1, 128:HID], w1_d[:, 0:1, 128:HID])
            feed = [1, 2, 2, 2, 2, 2, 4, 4, 4]
            c = 1
            for w in feed:
                w = min(w, KT_IN - c)
                if w <= 0:
                    break
                nc.sync.dma_start(XT[:, c : c + w, :], xt_d[:, c : c + w, :])
                nc.scalar.dma_start(W1[:, c : c + w, :], w1_d[:, c : c + w, :])
                c += w
            # Remaining weights behind the layer-1 feed on both rings.
            nc.sync.dma_start(W2[:], w2_d[:])
            nc.scalar.dma_start(W3[:], w3_d[:])
            nc.scalar.dma_start(W4[:], w4_d[:])

            def mm_pair(ps, Wk, rhs, k, kt):
                # The redundant second-half LDWEIGHTS is removed post-compile
                # by _dedup_ldweights (the in-IR ldweights flag is overwritten
                # by the rust split pass, so mutating it here does nothing).
                for idx, (s, w) in enumerate(nhalves):
                    nc.tensor.matmul(
                        ps[:, s : s + w],
                        Wk,
                        rhs[:, k, s : s + w],
                        start=(k == 0),
                        stop=(k == kt - 1),
                    )

            def relu_square(ps, jt, S, j, n):
                # relu: PSUM f32 -> SBUF bf16 on ScalarE; accum = batch sum.
                # sum of squares on VectorE from HRAW (hardware allows only
                # ONE psum operand per DVE op, so it can't read ps twice);
                # Vector is otherwise light, so all squares live there and
                # pipeline one tile behind the relus.
                nc.scalar.activation(
                    HRAW[:, jt, :], ps[:], RELU,
                    accum_out=S[:, j : j + 1],
                )
                scr = scrpool.tile([128, BL], BF16, tag="scr")
                nc.vector.scalar_tensor_tensor(
                    scr[:], HRAW[:, jt, :], 0.0, HRAW[:, jt, :],
                    mybir.AluOpType.bypass, MUL,
                    accum_out=S[:, n + j : n + j + 1],
                )

            def bn_ar_start(li, h, S, n):
                """Stats out + AllGather trigger for one feature group.

                Both ride the GpSimd queue, ordered so a group's stats
                export always issues before any earlier group's collective
                wait — exports are never stuck behind an unfinished
                collective. AllGather + local tree-sum beats AllReduce
                here: its mesh program is ~2x shorter on the serial CC
                engine, which is the scarce resource."""
                cc_in = dpool.tile([128, 2 * n], F32, tag="cc_in",
                                   name=f"cc_in_{li}_{h}")
                cc_out = dpool.tile([NCORES * 128, 2 * n], F32, tag="cc_out",
                                    name=f"cc_out_{li}_{h}")
                nc.gpsimd.dma_start(cc_in[:], S[:])
                nc.gpsimd.collective_compute(
                    "AllGather",
                    mybir.AluOpType.bypass,
                    replica_groups=[list(range(NCORES))],
                    ins=[cc_in.opt()],
                    outs=[cc_out.opt()],
                )
                return cc_out

            def bn_finish(li, h, jts, cc_out, Hdst, apply_out=True):
                """Pull gathered stats, compute a/c for `jts`, apply to Hdst.

                Split across queues so nothing upstream is ever gated by the
                collective wait: the gather-in rides the Sync ring (idle
                once the feed drains — on the gpsimd/CC ring it queues
                behind the NEXT collective's internal data movement and
                lands ~25us late); the core-sum reduce + stats math ride
                Vector; the one Sqrt rides Scalar between relu batches."""
                n = len(jts)
                # Gather [part, core, stat]: per-core contiguous runs keep
                # the DMA a fat 2D pattern. (A [part, stat, core] transpose
                # would allow a single innermost reduce, but lowers to an
                # element-strided DMA that takes ~17us — measured.)
                GAT = spool.tile([128, NCORES, 2 * n], F32, tag="GAT",
                                 name=f"GAT_{li}_{h}")
                nc.sync.dma_start(
                    GAT[:], cc_out.opt().rearrange("(c p) s -> p c s", p=128)
                )
                T4 = spool.tile([128, 4, 2 * n], F32, tag="T4", name=f"T4_{li}_{h}")
                nc.vector.tensor_tensor(T4[:], GAT[:, 0:4, :], GAT[:, 4:8, :], ADD)
                T2 = spool.tile([128, 2, 2 * n], F32, tag="T2", name=f"T2_{li}_{h}")
                nc.vector.tensor_tensor(T2[:], T4[:, 0:2, :], T4[:, 2:4, :], ADD)
                SS = spool.tile([128, 2 * n], F32, tag="SS", name=f"SS_{li}_{h}")
                nc.vector.tensor_tensor(SS[:], T2[:, 0, :], T2[:, 1, :], ADD)
                # a = g*rsqrt(var); c = beta - a*mean, computed scale-free:
                # V = E2 - E1^2/B = B*var (EPS dropped: shifts rsqrt ~5e-6
                # rel, far below the 2e-2 budget); host pre-scales gamma by
                # sqrt(B) so a = g' * sqrt(1/(B*var)) — the MEAN/MSQ ops of
                # the old chain are gone entirely.
                T = spool.tile([128, n], F32, tag="T", name=f"T_{li}_{h}")
                nc.vector.scalar_tensor_tensor(
                    T[:], SS[:, 0:n], 1.0 / B, SS[:, 0:n], MUL, MUL
                )
                V = spool.tile([128, n], F32, tag="V", name=f"V_{li}_{h}")
                nc.vector.tensor_tensor(V[:], SS[:, n : 2 * n], T[:], SUB)
                RI = spool.tile([128, n], F32, tag="RI", name=f"RI_{li}_{h}")
                nc.vector.reciprocal(RI[:], V[:])
                R = spool.tile([128, n], F32, tag="R", name=f"R_{li}_{h}")
                nc.scalar.activation(R[:], RI[:], SQRT)
                g0 = (2 * li) * JT + jts[0]
                b0 = (2 * li + 1) * JT + jts[0]
                A = spool.tile([128, n], F32, tag="A", name=f"A_{li}_{h}")
                nc.vector.tensor_tensor(A[:], R[:], BNP[:, g0 : g0 + n], MUL)
                AM = spool.tile([128, n], F32, tag="AM", name=f"AM_{li}_{h}")
                nc.vector.scalar_tensor_tensor(
                    AM[:], SS[:, 0:n], 1.0 / B, A[:], MUL, MUL
                )
                C = spool.tile([128, n], F32, tag="C", name=f"C_{li}_{h}")
                nc.vector.tensor_tensor(C[:], BNP[:, b0 : b0 + n], AM[:], SUB)
                if apply_out:
                    for j, jt in enumerate(jts):
                        nc.vector.tensor_scalar(
                            Hdst[:, jt, :],
                            HRAW[:, jt, :],
                            A[:, j : j + 1],
                            C[:, j : j + 1],
                            MUL,
                            ADD,
                        )
                return A, C

            def mlp_layer(li, kt, rhs, W, Hdst, groups, apply_out=True):
                """One layer: matmuls + relu + distributed BN into Hdst.

                The first 4 feature tiles run k-outer (so layer-1 consumes
                input tiles in DMA-arrival order), the rest as sequential
                chains. `groups` partitions the 8 tiles into stats
                AllGathers: ncfw runs its first ~3 collectives slowly
                (12-27us) and later ones at ~5us, so layer 1 uses [6,2]
                (fewest slow-era ops) while layers 2-3 use [4,4] (group-0
                applies land before the layer ends; only one trailing
                mesh). Each finish(g) is issued right after ar_start(g):
                its Vector math sits before the NEXT group's squares and
                its Scalar sqrt between relu batches, so stats production
                is never queued behind a collective-dependent op, while
                applies land as early as dependencies allow."""
                done = 0
                acs = []
                for gi, n in enumerate(groups):
                    jts = list(range(done, done + n))
                    done += n
                    S = spool.tile([128, 2 * n], F32, tag=f"S_g{gi}",
                                   name=f"S{li}_{gi}")
                    for i, jt in enumerate(jts):
                        if jt == 0:
                            # tiles 0-3: k-outer over 4 concurrent chains
                            pss = [
                                pspool.tile([128, BL], F32, tag="ps",
                                            name=f"ps_g{j}")
                                for j in range(4)
                            ]
                            for k in range(kt):
                                for j in range(4):
                                    mm_pair(
                                        pss[j], W[:, k, j * 128 : (j + 1) * 128],
                                        rhs, k, kt,
                                    )
                        if jt < 4:
                            relu_square(pss[jt], jt, S, i, n)
                        else:
                            ps = pspool.tile([128, BL], F32, tag="ps",
                                             name=f"ps_s{jt}")
                            for k in range(kt):
                                mm_pair(
                                    ps, W[:, k, jt * 128 : (jt + 1) * 128],
                                    rhs, k, kt,
                                )
                            relu_square(ps, jt, S, i, n)
                    cc = bn_ar_start(li, gi, S, n)
                    acs.append(
                        (bn_finish(li, gi, jts, cc, Hdst, apply_out), jts)
                    )
                return acs

            # ---- layers ----
            H2 = apool.tile([128, JT, BL], BF16, tag="H2")
            # Group choice per layer: meshes cost ~22us END-TO-END FROM THEIR
            # START while ncfw background-init runs (first ~145us of the
            # kernel), ~5-7us after, and the CC engine runs them serially
            # (measured: an [8] single L1 mesh starting at 122 ended 144 —
            # no better than [6,2]'s second mesh, but it starved L2 of the
            # first 6 tiles). L1 [6,2]: g0 exports at ~91 (j0-5 done), mesh
            # ends ~114-121 delivering 6/8 k-tiles for L2's first phase;
            # g1 mesh ends ~140-146. L2 [4,2,2] / L3 [4,4]: warm-era meshes
            # pipeline behind the j4-7 chains; the k-outer j0-3 structure
            # consumes k ascending so the grouped-apply staircase never
            # head-of-line-blocks the PE queue.
            mlp_layer(0, KT_IN, XT, W1, H, groups=[6, 2])
            mlp_layer(1, JT, H, W2, H2, groups=[4, 4])
            l3 = mlp_layer(2, JT, H2, W3, H, groups=[4, 4], apply_out=False)

            # ---- layer 4 (no relu/BN) ----
            # BN3 is folded into L4's weights instead of applied to H:
            #   out_j = sum_f W4[j,f] (a_f r_f + c_f)
            #         = sum_f (W4[j,f] a_f) r_f  +  sum_f (W4[j,f] a_f)(c_f/a_f)
            # so per k-tile we scale W4 by a (per-partition, 16-wide — ~60ns)
            # and accumulate one extra 1-column matmul of c/a into a bias
            # psum, SHARING the scaled stationary weights (no extra
            # LDWEIGHTS after dedup). L4's matmuls start right after each
            # group's a/c lands — no serial H-apply chain on the tail.
            W4S = wpool.tile([128, JT, CLSP], BF16, tag="W4S")
            ps4 = pspool.tile([CLSP, BL], F32, tag="ps", name="ps4")
            psb = pspool.tile([CLSP, 1], F32, tag="ps", name="psb")
            for gi, ((A, C), jts) in enumerate(l3):
                n = len(jts)
                AR = spool.tile([128, n], F32, tag="AR", name=f"AR4_{gi}")
                nc.vector.reciprocal(AR[:], A[:])
                CA = spool.tile([128, n], BF16, tag="CA", name=f"CA4_{gi}")
                nc.vector.tensor_tensor(CA[:], C[:], AR[:], MUL)
                for i, k in enumerate(jts):
                    nc.vector.tensor_scalar_mul(
                        W4S[:, k, :], W4[:, k, :], A[:, i : i + 1]
                    )
                    mm_pair(ps4, W4S[:, k, :], HRAW, k, JT)
                    nc.tensor.matmul(
                        psb[:, 0:1],
                        W4S[:, k, :],
                        CA[:, i : i + 1],
                        start=(k == 0),
                        stop=(k == JT - 1),
                    )
            # Drain PSUM->SBUF in halves on two engines in parallel (each
            # fused with the +bias), then DMA each half on its own ring.
            KS = spool.tile([CLSP, 1], F32, tag="KS")
            nc.scalar.copy(KS[:], psb[:])
            OUTS = spool.tile([CLSP, BL], F32, tag="OUTS")
            nc.scalar.activation(
                OUTS[:, 0:512], ps4[:, 0:512],
                mybir.ActivationFunctionType.Identity, bias=KS[:, 0:1],
            )
            nc.vector.tensor_scalar_add(OUTS[:, 512:BL], ps4[:, 512:BL], KS[:, 0:1])
            nc.sync.dma_start(out_d[:, 0:512], OUTS[:, 0:512])
            nc.scalar.dma_start(out_d[:, 512:BL], OUTS[:, 512:BL])
            # Warmup-output anchors (keep the warmup collectives from DCE).
            # Issued last so the scheduler can't wedge their long warmup-
            # waits into the middle of the input feed; on the GpSimd ring
            # (idle after the last stats trigger) so they run well before
            # the output DMAs instead of queuing behind them on Sync.
            for wi, wo in enumerate(wu_outs):
                nc.gpsimd.dma_start(
                    out_d[CLSP - 1 : CLSP, wi : wi + 1], wo[0:1, 0:1]
                )

    nc.compile()
    _dedup_ldweights(nc)
    return nc


def _get_nc():
    if "nc" not in _CACHE:
        _CACHE["nc"] = _build()
    return _CACHE["nc"]


def _prep_inputs(x, W1, W2, W3, W4, g1, b1, g2, b2, g3, b3):
    x2 = np.asarray(x, dtype=np.float32).reshape(B, KIN)
    xt = np.ascontiguousarray(x2.T).astype(ml_dtypes.bfloat16)  # [3072, 8192]

    def pmajor(a):
        # [ktiles*128, free] -> [128, ktiles, free] (partition-major)
        kt = a.shape[0] // 128
        return np.ascontiguousarray(
            a.reshape(kt, 128, a.shape[1]).transpose(1, 0, 2)
        )

    def bin_t(w, pad=None, dtype=ml_dtypes.float8_e4m3):
        wb = np.where(np.asarray(w, dtype=np.float32) >= 0, 1.0, -1.0)
        wt = np.ascontiguousarray(wb.T).astype(dtype)  # [in, out], +-1 exact
        if pad is not None and wt.shape[1] < pad:
            wt = np.concatenate(
                [wt, np.zeros((wt.shape[0], pad - wt.shape[1]), wt.dtype)], axis=1
            )
        return pmajor(wt)

    w1t = bin_t(W1)            # [128, 24, 1024]
    w2t = bin_t(W2)            # [128, 8, 1024]
    w3t = bin_t(W3)
    # W4 ships bf16: it gets rescaled by the BN3 "a" on-device (W4' trick)
    w4t = bin_t(W4, pad=CLSP, dtype=ml_dtypes.bfloat16)  # [128, 8, 16]

    bnp = np.zeros((128, 6 * JT), dtype=np.float32)
    for l, p in enumerate([g1, b1, g2, b2, g3, b3]):
        pa = np.asarray(p, dtype=np.float32)
        if l % 2 == 0:
            # gammas pre-scaled by sqrt(B): the device computes
            # a = g' * rsqrt(B*var) without ever forming var itself.
            pa = pa * np.sqrt(float(B))
        for jt in range(JT):
            bnp[:, l * JT + jt] = pa[jt * 128 : (jt + 1) * 128]

    shared = {"w1t": w1t, "w2t": w2t, "w3t": w3t, "w4t": w4t, "bnp": bnp}
    in_maps = []
    for c in range(NCORES):
        m = dict(shared)
        m["xt"] = pmajor(np.ascontiguousarray(xt[:, c * BL : (c + 1) * BL]))
        in_maps.append(m)
    return in_maps


def _run(inputs, trace=False):
    nc = _get_nc()
    in_maps = _prep_inputs(**inputs)
    res = bass_utils.run_bass_kernel_spmd(
        nc, in_maps, core_ids=list(range(NCORES)), trace=trace
    )
    out = np.empty((B, CLS), dtype=np.float32)
    for c in range(NCORES):
        out[c * BL : (c + 1) * BL, :] = res.results[c]["out"][:CLS, :].T
    return out, res


def kernel(**inputs):
    out, _ = _run(inputs, trace=False)
    return out



# revision 37
# speedup vs baseline: 1.0615x; 1.0615x over previous
"""Trainium2 Bass kernel for nn_BinaryLinear (binarized 4-layer MLP + BatchNorm).

Reference computation (fp32, jax):
    h = x.reshape(-1, 3072)
    h = relu(h @ sign(W1).T); h = BN(h, g1, b1)   # BN over full 8192 batch
    h = relu(h @ sign(W2).T); h = BN(h, g2, b2)
    h = relu(h @ sign(W3).T); h = BN(h, g3, b3)
    out = h @ sign(W4).T                          # [8192, 10]

Strategy (8 NeuronCores, data-parallel over batch; ~237-276us on silicon —
the spread is the ncfw collective-firmware wake, 60-105us run-to-run; the
first two stats meshes complete at wake_end+~40 and wake_end+~63 regardless
of when they start, and everything downstream is ~103us of stable work):
  - Host: binarize weights to fp8/bf16 (+-1 exact), pack partition-major
    ([128, ktiles, free]) so DMAs are fat 2D-contiguous transfers, shard x
    over cores (1024 rows each), pre-scale BN gammas by sqrt(B).
  - Device (SPMD identical program): activations live feature-major
    [feature_part, batch_free] in SBUF. Each layer is a K-tiled bf16
    matmul accumulating in PSUM; the first 4 feature tiles run k-outer
    (layer-1 consumes input tiles in DMA arrival order), the rest as
    sequential chains. Matmul cadence is ~263ns per 512-col MM (silicon
    floor; hiding LDWEIGHTS does not change it). Relu on ScalarE (batch
    sum free via accum_out); sum(relu^2) on VectorE (stt h*h).
  - BatchNorm over the full 8192 batch via one AllGather per feature
    group + local tree-sum (payload [128, 2n] f32; transposed gathers
    lower to element-strided DMA ~17us, keep [part, core, stat]). The CC
    engine is serial and meshes are slow until ncfw init settles, so
    groups are [6,2]/[4,4]/[4,4]: L1's 6-group exports as early as
    possible (its mesh ends ~wake+40, delivering 6/8 k-tiles for L2's
    first phase), later layers use warm ~6us meshes. Stats chain is
    var-free: V = E2 - E1^2/B, a = g*sqrt(B)*sqrt(1/V) (one reciprocal +
    one Sqrt; Rsqrt activation is blocked for accuracy). Queue
    discipline: exports + triggers on GpSimd, gather-ins on Sync, stats
    math + applies on Vector, Sqrt on Scalar. A warmup AllGather at
    kernel start absorbs the ncfw wake off the critical path; act tables
    (Relu/Sqrt/Identity) are warmed at t~10us so no mid-kernel load.
  - Layer 4 folds BN3 into the weights (W4' trick): out = sum_k
    (W4[k]*a_k) @ relu_raw[k] + bias, the bias accumulated via a 1-column
    matmul of c/a sharing the scaled stationary weights; L4's matmuls
    chase each stats group directly with no H-apply chain on the tail,
    and the PSUM->SBUF drain fuses the +bias on both engine halves.
  - Post-compile, _dedup_ldweights removes the 326 redundant second-half
    weight loads from the BIR (halves Tensor-queue pressure).
  - Feed: XT/W2 on the Sync HWDGE ring, W1/W3/W4 on the Scalar ring, in
    growing chunks so the first accumulation chains start early.
"""
import os
import sys

for _p in ("/opt/trn_rl_repo",):
    if os.path.isdir(_p) and _p not in sys.path:
        sys.path.insert(0, _p)

import numpy as np
import ml_dtypes

from concourse import bacc, tile, mybir
from concourse import bass_utils

NCORES = 8
B = 8192
BL = B // NCORES            # 1024 rows per core
KIN = 3072
KT_IN = KIN // 128          # 24 k-tiles for layer 1
HID = 1024
JT = HID // 128             # 8 feature tiles
CLS = 10
CLSP = 16                   # padded classes
EPS = 1e-5
BF16 = mybir.dt.bfloat16
FP8 = mybir.dt.float8e4   # +-1 weights are exact in fp8; halves weight DMA traffic
F32 = mybir.dt.float32
ADD = mybir.AluOpType.add
SUB = mybir.AluOpType.subtract
MUL = mybir.AluOpType.mult
RELU = mybir.ActivationFunctionType.Relu
SQRT = mybir.ActivationFunctionType.Sqrt
MAX = mybir.AluOpType.max

_CACHE = {}


def _dedup_ldweights(nc):
    """Drop redundant InstLdweights from the compiled BIR.

    The rust lowering splits every matmul into InstLdweights +
    InstMatmult(ldweights=False), one load per matmul, even when
    consecutive matmuls use the identical stationary tile (the two
    512-col halves of each K-tile pair). The exposed load costs ~50ns
    per matmul on the PE array (measured 263ns/MM vs 213ns streaming).
    Here we remove an InstLdweights when (a) the previous InstLdweights
    in the same block loaded the exact same weights (same lowered AP,
    perf_mode, tile_position, transpose), (b) no other array-state
    change sits between them (only InstLdweights writes the array;
    matmuls/semaphores don't), and (c) it carries no semaphore waits or
    updates (so dropping it cannot break synchronization -- the waits
    that matter were moved onto the FIRST load of each pair by
    move_matmul_waits_to_ldweights).
    """
    removed = kept = 0
    for b in nc.m.functions[0].blocks:
        insts = b.instructions
        drop = []
        prev_key = None
        for i, ins in enumerate(insts):
            if isinstance(ins, mybir.InstLdweights):
                key = (
                    str(ins.ins[0]),
                    str(ins.perf_mode),
                    str(ins.tile_position),
                    str(ins.is_transpose),
                )
                si = ins.sync_info
                clean = si is None or (
                    len(si.on_wait) == 0 and len(si.on_update) == 0
                )
                if key == prev_key and clean:
                    drop.append(i)
                    removed += 1
                else:
                    kept += 1
                prev_key = key
        for i in reversed(drop):
            del insts[i]
    return removed, kept


def _build():
    nc = bacc.Bacc("TRN2", target_bir_lowering=False, debug=False, num_devices=NCORES)

    # All bulk inputs are partition-major on the host ([128, ktiles, free])
    # so DMAs are cheap-descriptor 2D patterns at full bandwidth.
    xt_d = nc.dram_tensor("xt", [128, KT_IN, BL], BF16, kind="ExternalInput")
    w1_d = nc.dram_tensor("w1t", [128, KT_IN, HID], FP8, kind="ExternalInput")
    w2_d = nc.dram_tensor("w2t", [128, JT, HID], FP8, kind="ExternalInput")
    w3_d = nc.dram_tensor("w3t", [128, JT, HID], FP8, kind="ExternalInput")
    w4_d = nc.dram_tensor("w4t", [128, JT, CLSP], BF16, kind="ExternalInput")
    bnp_d = nc.dram_tensor("bnp", [128, 6 * JT], F32, kind="ExternalInput")
    out_d = nc.dram_tensor("out", [CLSP, BL], F32, kind="ExternalOutput")

    nhalves = [(s, min(512, BL - s)) for s in range(0, BL, 512)]

    with tile.TileContext(nc) as tc:
        with (
            tc.tile_pool(name="weights", bufs=1) as wpool,
            tc.tile_pool(name="acts", bufs=1) as apool,
            tc.tile_pool(name="scratch", bufs=2) as scrpool,
            tc.tile_pool(name="stats", bufs=2) as spool,
            tc.tile_pool(name="psum", bufs=4, space="PSUM") as pspool,
            tc.tile_pool(name="dram", bufs=2, space="DRAM") as dpool,
        ):
            XT = wpool.tile([128, KT_IN, BL], BF16, tag="XT")
            W1 = wpool.tile([128, KT_IN, HID], FP8, tag="W1")
            W2 = wpool.tile([128, JT, HID], FP8, tag="W2")
            W3 = wpool.tile([128, JT, HID], FP8, tag="W3")
            W4 = wpool.tile([128, JT, CLSP], BF16, tag="W4")
            BNP = wpool.tile([128, 6 * JT], F32, tag="BNP")
            HRAW = apool.tile([128, JT, BL], BF16, tag="HRAW")
            H = apool.tile([128, JT, BL], BF16, tag="H")

            # One warmup collective: absorbs the ncfw wake + init cost
            # (~60-110us, run-variable) off the critical path. Input is an
            # unwritten scratch buffer (contents irrelevant); output
            # anchored into an unused out_d row at program end so DCE
            # keeps it. Collective latency stays elevated (~12-40us/mesh,
            # environmental) until ncfw's background init finishes around
            # 150us — extra or shape-matched warmups were measured NOT to
            # help, so keep exactly one and minimize early-era collectives.
            wu_outs = []
            for wi, cols in enumerate((1,)):
                win = dpool.tile([128, cols], F32, tag=f"wu_in{wi}")
                wo = dpool.tile([NCORES * 128, cols], F32, tag=f"wu_out{wi}")
                nc.gpsimd.collective_compute(
                    "AllGather",
                    mybir.AluOpType.bypass,
                    replica_groups=[list(range(NCORES))],
                    ins=[win.opt()],
                    outs=[wo.opt()],
                )
                wu_outs.append(wo)

            # Warm the Scalar activation tables (Relu + Rsqrt) at program
            # start so no ACT_TABLE_LOAD (~1.3us) lands mid-kernel on the
            # stats critical path. Input is uninitialized scratch (values
            # irrelevant, output discarded).
            WARM = scrpool.tile([128, 1], F32, tag="warm")
            WARMO = scrpool.tile([128, 1], F32, tag="warmo")
            nc.vector.memset(WARM[:], 1.0)
            nc.scalar.activation(WARMO[:], WARM[:], RELU)
            nc.scalar.activation(WARMO[:], WARM[:], SQRT)
            nc.scalar.activation(
                WARMO[:], WARM[:], mybir.ActivationFunctionType.Identity
            )

            # Input feed: XT on the Sync HWDGE ring, W1 on the Scalar HWDGE
            # ring, in progressively larger chunks so the first accumulation
            # chains start early while the bulk still moves in fat transfers.
            nc.sync.dma_start(BNP[:], bnp_d[:])
            # Tiny first transfers so the very first matmul (k=0, j=0,
            # batch half 0) can start ~1.5us earlier, then the bulk feed.
            nc.sync.dma_start(XT[:, 0:1, 0:512], xt_d[:, 0:1, 0:512])
            nc.scalar.dma_start(W1[:, 0:1, 0:128], w1_d[:, 0:1, 0:128])
            nc.sync.dma_start(XT[:, 0:1, 512:BL], xt_d[:, 0:1, 512:BL])
            nc.scalar.dma_start(W1[:, 0:1, 128:HID], w1_d[:, 0:1, 128:HID])
            feed = [1, 2, 2, 2, 2, 2, 4, 4, 4]
            c = 1
            for w in feed:
                w = min(w, KT_IN - c)
                if w <= 0:
                    break
                nc.sync.dma_start(XT[:, c : c + w, :], xt_d[:, c : c + w, :])
                nc.scalar.dma_start(W1[:, c : c + w, :], w1_d[:, c : c + w, :])
                c += w
            # Remaining weights behind the layer-1 feed on both rings.
            nc.sync.dma_start(W2[:], w2_d[:])
            nc.scalar.dma_start(W3[:], w3_d[:])
            nc.scalar.dma_start(W4[:], w4_d[:])

            def mm_pair(ps, Wk, rhs, k, kt):
                # The redundant second-half LDWEIGHTS is removed post-compile
                # by _dedup_ldweights (the in-IR ldweights flag is overwritten
                # by the rust split pass, so mutating it here does nothing).
                for idx, (s, w) in enumerate(nhalves):
                    nc.tensor.matmul(
                        ps[:, s : s + w],
                        Wk,
                        rhs[:, k, s : s + w],
                        start=(k == 0),
                        stop=(k == kt - 1),
                    )

            def relu_square(ps, jt, S, j, n):
                # relu: PSUM f32 -> SBUF bf16 on ScalarE; accum = batch sum.
                # sum of squares on VectorE from HRAW (hardware allows only
                # ONE psum operand per DVE op, so it can't read ps twice);
                # Vector is otherwise light, so all squares live there and
                # pipeline one tile behind the relus.
                nc.scalar.activation(
                    HRAW[:, jt, :], ps[:], RELU,
                    accum_out=S[:, j : j + 1],
                )
                scr = scrpool.tile([128, BL], BF16, tag="scr")
                nc.vector.scalar_tensor_tensor(
                    scr[:], HRAW[:, jt, :], 0.0, HRAW[:, jt, :],
                    mybir.AluOpType.bypass, MUL,
                    accum_out=S[:, n + j : n + j + 1],
                )

            def bn_ar_start(li, h, S, n):
                """Stats out + AllGather trigger for one feature group.

                Both ride the GpSimd queue, ordered so a group's stats
                export always issues before any earlier group's collective
                wait — exports are never stuck behind an unfinished
                collective. AllGather + local tree-sum beats AllReduce
                here: its mesh program is ~2x shorter on the serial CC
                engine, which is the scarce resource."""
                cc_in = dpool.tile([128, 2 * n], F32, tag="cc_in",
                                   name=f"cc_in_{li}_{h}")
                cc_out = dpool.tile([NCORES * 128, 2 * n], F32, tag="cc_out",
                                    name=f"cc_out_{li}_{h}")
                nc.gpsimd.dma_start(cc_in[:], S[:])
                nc.gpsimd.collective_compute(
                    "AllGather",
                    mybir.AluOpType.bypass,
                    replica_groups=[list(range(NCORES))],
                    ins=[cc_in.opt()],
                    outs=[cc_out.opt()],
                )
                return cc_out

            def bn_finish(li, h, jts, cc_out, Hdst, apply_out=True):
                """Pull gathered stats, compute a/c for `jts`, apply to Hdst.

                Split across queues so nothing upstream is ever gated by the
                collective wait: the gather-in rides the Sync ring (idle
                once the feed drains — on the gpsimd/CC ring it queues
                behind the NEXT collective's internal data movement and
                lands ~25us late); the core-sum reduce + stats math ride
                Vector; the one Sqrt rides Scalar between relu batches."""
                n = len(jts)
                # Gather [part, core, stat]: per-core contiguous runs keep
                # the DMA a fat 2D pattern. (A [part, stat, core] transpose
                # would allow a single innermost reduce, but lowers to an
                # element-strided DMA that takes ~17us — measured.)
                GAT = spool.tile([128, NCORES, 2 * n], F32, tag="GAT",
                                 name=f"GAT_{li}_{h}")
                nc.sync.dma_start(
                    GAT[:], cc_out.opt().rearrange("(c p) s -> p c s", p=128)
                )
                T4 = spool.tile([128, 4, 2 * n], F32, tag="T4", name=f"T4_{li}_{h}")
                nc.vector.tensor_tensor(T4[:], GAT[:, 0:4, :], GAT[:, 4:8, :], ADD)
                T2 = spool.tile([128, 2, 2 * n], F32, tag="T2", name=f"T2_{li}_{h}")
                nc.vector.tensor_tensor(T2[:], T4[:, 0:2, :], T4[:, 2:4, :], ADD)
                SS = spool.tile([128, 2 * n], F32, tag="SS", name=f"SS_{li}_{h}")
                nc.vector.tensor_tensor(SS[:], T2[:, 0, :], T2[:, 1, :], ADD)
                # a = g*rsqrt(var); c = beta - a*mean, computed scale-free:
                # V = E2 - E1^2/B = B*var (EPS dropped: shifts rsqrt ~5e-6
                # rel, far below the 2e-2 budget); host pre-scales gamma by
                # sqrt(B) so a = g' * sqrt(1/(B*var)) — the MEAN/MSQ ops of
                # the old chain are gone entirely.
                T = spool.tile([128, n], F32, tag="T", name=f"T_{li}_{h}")
                nc.vector.scalar_tensor_tensor(
                    T[:], SS[:, 0:n], 1.0 / B, SS[:, 0:n], MUL, MUL
                )
                V = spool.tile([128, n], F32, tag="V", name=f"V_{li}_{h}")
                nc.vector.tensor_tensor(V[:], SS[:, n : 2 * n], T[:], SUB)
                RI = spool.tile([128, n], F32, tag="RI", name=f"RI_{li}_{h}")
                nc.vector.reciprocal(RI[:], V[:])
                R = spool.tile([128, n], F32, tag="R", name=f"R_{li}_{h}")
                nc.scalar.activation(R[:], RI[:], SQRT)
                g0 = (2 * li) * JT + jts[0]
                b0 = (2 * li + 1) * JT + jts[0]
                A = spool.tile([128, n], F32, tag="A", name=f"A_{li}_{h}")
                nc.vector.tensor_tensor(A[:], R[:], BNP[:, g0 : g0 + n], MUL)
                AM = spool.tile([128, n], F32, tag="AM", name=f"AM_{li}_{h}")
                nc.vector.scalar_tensor_tensor(
                    AM[:], SS[:, 0:n], 1.0 / B, A[:], MUL, MUL
                )
                C = spool.tile([128, n], F32, tag="C", name=f"C_{li}_{h}")
                nc.vector.tensor_tensor(C[:], BNP[:, b0 : b0 + n], AM[:], SUB)
                if apply_out:
                    # Alternate the per-tile applies across VectorE and
                    # ScalarE (activation Identity computes scale*x+bias with
                    # per-partition APs) so consecutive tiles release in
                    # ~half the wall time — the next layer's first chains
                    # unblock sooner after each mesh.
                    for j, jt in enumerate(jts):
                        if j % 2 == 0:
                            nc.vector.tensor_scalar(
                                Hdst[:, jt, :],
                                HRAW[:, jt, :],
                                A[:, j : j + 1],
                                C[:, j : j + 1],
                                MUL,
                                ADD,
                            )
                        else:
                            nc.scalar.activation(
                                Hdst[:, jt, :],
                                HRAW[:, jt, :],
                                mybir.ActivationFunctionType.Identity,
                                bias=C[:, j : j + 1],
                                scale=A[:, j : j + 1],
                            )
                return A, C

            def mlp_layer(li, kt, rhs, W, Hdst, groups, apply_out=True):
                """One layer: matmuls + relu + distributed BN into Hdst.

                The first 4 feature tiles run k-outer (so layer-1 consumes
                input tiles in DMA-arrival order), the rest as sequential
                chains. `groups` partitions the 8 tiles into stats
                AllGathers: ncfw runs its first ~3 collectives slowly
                (12-27us) and later ones at ~5us, so layer 1 uses [6,2]
                (fewest slow-era ops) while layers 2-3 use [4,4] (group-0
                applies land before the layer ends; only one trailing
                mesh). Each finish(g) is issued right after ar_start(g):
                its Vector math sits before the NEXT group's squares and
                its Scalar sqrt between relu batches, so stats production
                is never queued behind a collective-dependent op, while
                applies land as early as dependencies allow."""
                done = 0
                acs = []
                for gi, n in enumerate(groups):
                    jts = list(range(done, done + n))
                    done += n
                    S = spool.tile([128, 2 * n], F32, tag=f"S_g{gi}",
                                   name=f"S{li}_{gi}")
                    for i, jt in enumerate(jts):
                        if jt == 0:
                            # tiles 0-3: k-outer over 4 concurrent chains
                            pss = [
                                pspool.tile([128, BL], F32, tag="ps",
                                            name=f"ps_g{j}")
                                for j in range(4)
                            ]
                            for k in range(kt):
                                for j in range(4):
                                    mm_pair(
                                        pss[j], W[:, k, j * 128 : (j + 1) * 128],
                                        rhs, k, kt,
                                    )
                        if jt < 4:
                            relu_square(pss[jt], jt, S, i, n)
                        else:
                            ps = pspool.tile([128, BL], F32, tag="ps",
                                             name=f"ps_s{jt}")
                            for k in range(kt):
                                mm_pair(
                                    ps, W[:, k, jt * 128 : (jt + 1) * 128],
                                    rhs, k, kt,
                                )
                            relu_square(ps, jt, S, i, n)
                    cc = bn_ar_start(li, gi, S, n)
                    acs.append(
                        (bn_finish(li, gi, jts, cc, Hdst, apply_out), jts)
                    )
                return acs

            # ---- layers ----
            H2 = apool.tile([128, JT, BL], BF16, tag="H2")
            # Group choice per layer: meshes cost ~22us END-TO-END FROM THEIR
            # START while ncfw background-init runs (first ~145us of the
            # kernel), ~5-7us after, and the CC engine runs them serially
            # (measured: an [8] single L1 mesh starting at 122 ended 144 —
            # no better than [6,2]'s second mesh, but it starved L2 of the
            # first 6 tiles). L1 [6,2]: g0 exports at ~91 (j0-5 done), mesh
            # ends ~114-121 delivering 6/8 k-tiles for L2's first phase;
            # g1 mesh ends ~140-146. L2 [4,2,2] / L3 [4,4]: warm-era meshes
            # pipeline behind the j4-7 chains; the k-outer j0-3 structure
            # consumes k ascending so the grouped-apply staircase never
            # head-of-line-blocks the PE queue.
            mlp_layer(0, KT_IN, XT, W1, H, groups=[6, 2])
            mlp_layer(1, JT, H, W2, H2, groups=[4, 4])
            # L3 [6,2]: the LAST group gates the kernel's end-chain (its mesh
            # + stats + the L4 k-tiles it unlocks) — keep it to 2 tiles so
            # the final chain is 4 main + 2 bias matmuls, while the [6]
            # group's mesh completes during L3's own j6/j7 chains and lets
            # L4 chew k0-5 (warm, behind the keep-warm dummies) early.
            l3 = mlp_layer(2, JT, H2, W3, H, groups=[6, 2], apply_out=False)

            # ---- layer 4 (no relu/BN) ----
            # BN3 is folded into L4's weights instead of applied to H:
            #   out_j = sum_f W4[j,f] (a_f r_f + c_f)
            #         = sum_f (W4[j,f] a_f) r_f  +  sum_f (W4[j,f] a_f)(c_f/a_f)
            # so per k-tile we scale W4 by a (per-partition, 16-wide — ~60ns)
            # and accumulate one extra 1-column matmul of c/a into a bias
            # psum, SHARING the scaled stationary weights (no extra
            # LDWEIGHTS after dedup). L4's matmuls start right after each
            # group's a/c lands — no serial H-apply chain on the tail.
            W4S = wpool.tile([128, JT, CLSP], BF16, tag="W4S")
            ps4 = pspool.tile([CLSP, BL], F32, tag="ps", name="ps4")
            psb = pspool.tile([CLSP, 1], F32, tag="ps", name="psb")
            # Keep-warm: the PE idles ~5-9us at each L3 stats-mesh wait and
            # HAM throttles it back to the 1.2GHz p-state (>3us idle), making
            # L4's matmuls run at ~427ns instead of ~263ns. Fill the two
            # known idle windows with dummy matmuls on resident data (same
            # stationary tile -> single deduped LDWEIGHTS; psum never read).
            def keep_warm(tag, pairs):
                psw = pspool.tile([128, 512], F32, tag="ps", name=tag)
                for _ in range(pairs):
                    nc.tensor.matmul(
                        psw[:, 0:512],
                        W3[:, 0, 0:128],
                        H2[:, 0, 0:512],
                        start=True,
                        stop=True,
                    )

            keep_warm("ps_kw0", 8)
            for gi, ((A, C), jts) in enumerate(l3):
                if gi == 1:
                    keep_warm("ps_kw1", 12)
                n = len(jts)
                AR = spool.tile([128, n], F32, tag="AR", name=f"AR4_{gi}")
                nc.vector.reciprocal(AR[:], A[:])
                CA = spool.tile([128, n], BF16, tag="CA", name=f"CA4_{gi}")
                nc.vector.tensor_tensor(CA[:], C[:], AR[:], MUL)
                for i, k in enumerate(jts):
                    nc.vector.tensor_scalar_mul(
                        W4S[:, k, :], W4[:, k, :], A[:, i : i + 1]
                    )
                    # bias first: psb finishes before the last main pair, so
                    # the KS copy overlaps the final matmuls.
                    nc.tensor.matmul(
                        psb[:, 0:1],
                        W4S[:, k, :],
                        CA[:, i : i + 1],
                        start=(k == 0),
                        stop=(k == JT - 1),
                    )
                    mm_pair(ps4, W4S[:, k, :], HRAW, k, JT)
            # Drain PSUM->SBUF in halves on two engines in parallel (each
            # fused with the +bias), then DMA each half on its own ring.
            KS = spool.tile([CLSP, 1], F32, tag="KS")
            nc.scalar.copy(KS[:], psb[:])
            OUTS = spool.tile([CLSP, BL], F32, tag="OUTS")
            nc.scalar.activation(
                OUTS[:, 0:512], ps4[:, 0:512],
                mybir.ActivationFunctionType.Identity, bias=KS[:, 0:1],
            )
            nc.vector.tensor_scalar_add(OUTS[:, 512:BL], ps4[:, 512:BL], KS[:, 0:1])
            nc.sync.dma_start(out_d[:, 0:512], OUTS[:, 0:512])
            nc.scalar.dma_start(out_d[:, 512:BL], OUTS[:, 512:BL])
            # Warmup-output anchors (keep the warmup collectives from DCE).
            # Issued last so the scheduler can't wedge their long warmup-
            # waits into the middle of the input feed; on the GpSimd ring
            # (idle after the last stats trigger) so they run well before
            # the output DMAs instead of queuing behind them on Sync.
            for wi, wo in enumerate(wu_outs):
                nc.gpsimd.dma_start(
                    out_d[CLSP - 1 : CLSP, wi : wi + 1], wo[0:1, 0:1]
                )

    nc.compile()
    _dedup_ldweights(nc)
    return nc


def _get_nc():
    if "nc" not in _CACHE:
        _CACHE["nc"] = _build()
    return _CACHE["nc"]


def _prep_inputs(x, W1, W2, W3, W4, g1, b1, g2, b2, g3, b3):
    x2 = np.asarray(x, dtype=np.float32).reshape(B, KIN)
    xt = np.ascontiguousarray(x2.T).astype(ml_dtypes.bfloat16)  # [3072, 8192]

    def pmajor(a):
        # [ktiles*128, free] -> [128, ktiles, free] (partition-major)
        kt = a.shape[0] // 128
        return np.ascontiguousarray(
            a.reshape(kt, 128, a.shape[1]).transpose(1, 0, 2)
        )

    def bin_t(w, pad=None, dtype=ml_dtypes.float8_e4m3):
        wb = np.where(np.asarray(w, dtype=np.float32) >= 0, 1.0, -1.0)
        wt = np.ascontiguousarray(wb.T).astype(dtype)  # [in, out], +-1 exact
        if pad is not None and wt.shape[1] < pad:
            wt = np.concatenate(
                [wt, np.zeros((wt.shape[0], pad - wt.shape[1]), wt.dtype)], axis=1
            )
        return pmajor(wt)

    w1t = bin_t(W1)            # [128, 24, 1024]
    w2t = bin_t(W2)            # [128, 8, 1024]
    w3t = bin_t(W3)
    # W4 ships bf16: it gets rescaled by the BN3 "a" on-device (W4' trick)
    w4t = bin_t(W4, pad=CLSP, dtype=ml_dtypes.bfloat16)  # [128, 8, 16]

    bnp = np.zeros((128, 6 * JT), dtype=np.float32)
    for l, p in enumerate([g1, b1, g2, b2, g3, b3]):
        pa = np.asarray(p, dtype=np.float32)
        if l % 2 == 0:
            # gammas pre-scaled by sqrt(B): the device computes
            # a = g' * rsqrt(B*var) without ever forming var itself.
            pa = pa * np.sqrt(float(B))
        for jt in range(JT):
            bnp[:, l * JT + jt] = pa[jt * 128 : (jt + 1) * 128]

    shared = {"w1t": w1t, "w2t": w2t, "w3t": w3t, "w4t": w4t, "bnp": bnp}
    in_maps = []
    for c in range(NCORES):
        m = dict(shared)
        m["xt"] = pmajor(np.ascontiguousarray(xt[:, c * BL : (c + 1) * BL]))
        in_maps.append(m)
    return in_maps


def _run(inputs, trace=False):
    nc = _get_nc()
    in_maps = _prep_inputs(**inputs)
    res = bass_utils.run_bass_kernel_spmd(
        nc, in_maps, core_ids=list(range(NCORES)), trace=trace
    )
    out = np.empty((B, CLS), dtype=np.float32)
    for c in range(NCORES):
        out[c * BL : (c + 1) * BL, :] = res.results[c]["out"][:CLS, :].T
    return out, res


def kernel(**inputs):
    out, _ = _run(inputs, trace=False)
    return out



# revision 39
# speedup vs baseline: 1.1273x; 1.0620x over previous
"""Trainium2 Bass kernel for nn_BinaryLinear (binarized 4-layer MLP + BatchNorm).

Reference computation (fp32, jax):
    h = x.reshape(-1, 3072)
    h = relu(h @ sign(W1).T); h = BN(h, g1, b1)   # BN over full 8192 batch
    h = relu(h @ sign(W2).T); h = BN(h, g2, b2)
    h = relu(h @ sign(W3).T); h = BN(h, g3, b3)
    out = h @ sign(W4).T                          # [8192, 10]

Strategy (8 NeuronCores, data-parallel over batch; ~237-276us on silicon —
the spread is the ncfw collective-firmware wake, 60-105us run-to-run; the
first two stats meshes complete at wake_end+~40 and wake_end+~63 regardless
of when they start, and everything downstream is ~103us of stable work):
  - Host: binarize weights to fp8/bf16 (+-1 exact), pack partition-major
    ([128, ktiles, free]) so DMAs are fat 2D-contiguous transfers, shard x
    over cores (1024 rows each), pre-scale BN gammas by sqrt(B).
  - Device (SPMD identical program): activations live feature-major
    [feature_part, batch_free] in SBUF. Each layer is a K-tiled bf16
    matmul accumulating in PSUM; the first 4 feature tiles run k-outer
    (layer-1 consumes input tiles in DMA arrival order), the rest as
    sequential chains. Matmul cadence is ~263ns per 512-col MM (silicon
    floor; hiding LDWEIGHTS does not change it). Relu on ScalarE (batch
    sum free via accum_out); sum(relu^2) on VectorE (stt h*h).
  - BatchNorm over the full 8192 batch via one AllGather per feature
    group + local tree-sum (payload [128, 2n] f32; transposed gathers
    lower to element-strided DMA ~17us, keep [part, core, stat]). The CC
    engine is serial and meshes are slow until ncfw init settles, so
    groups are [6,2]/[4,4]/[4,4]: L1's 6-group exports as early as
    possible (its mesh ends ~wake+40, delivering 6/8 k-tiles for L2's
    first phase), later layers use warm ~6us meshes. Stats chain is
    var-free: V = E2 - E1^2/B, a = g*sqrt(B)*sqrt(1/V) (one reciprocal +
    one Sqrt; Rsqrt activation is blocked for accuracy). Queue
    discipline: exports + triggers on GpSimd, gather-ins on Sync, stats
    math + applies on Vector, Sqrt on Scalar. A warmup AllGather at
    kernel start absorbs the ncfw wake off the critical path; act tables
    (Relu/Sqrt/Identity) are warmed at t~10us so no mid-kernel load.
  - Layer 4 folds BN3 into the weights (W4' trick): out = sum_k
    (W4[k]*a_k) @ relu_raw[k] + bias, the bias accumulated via a 1-column
    matmul of c/a sharing the scaled stationary weights; L4's matmuls
    chase each stats group directly with no H-apply chain on the tail,
    and the PSUM->SBUF drain fuses the +bias on both engine halves.
  - Post-compile, _dedup_ldweights removes the 326 redundant second-half
    weight loads from the BIR (halves Tensor-queue pressure).
  - Feed: XT/W2 on the Sync HWDGE ring, W1/W3/W4 on the Scalar ring, in
    growing chunks so the first accumulation chains start early.
"""
import os
import sys

for _p in ("/opt/trn_rl_repo",):
    if os.path.isdir(_p) and _p not in sys.path:
        sys.path.insert(0, _p)

import numpy as np
import ml_dtypes

from concourse import bacc, tile, mybir
from concourse import bass_utils

NCORES = 8
B = 8192
BL = B // NCORES            # 1024 rows per core
KIN = 3072
KT_IN = KIN // 128          # 24 k-tiles for layer 1
HID = 1024
JT = HID // 128             # 8 feature tiles
CLS = 10
CLSP = 16                   # padded classes
EPS = 1e-5
BF16 = mybir.dt.bfloat16
FP8 = mybir.dt.float8e4   # +-1 weights are exact in fp8; halves weight DMA traffic
F32 = mybir.dt.float32
ADD = mybir.AluOpType.add
SUB = mybir.AluOpType.subtract
MUL = mybir.AluOpType.mult
RELU = mybir.ActivationFunctionType.Relu
SQRT = mybir.ActivationFunctionType.Sqrt
MAX = mybir.AluOpType.max

_CACHE = {}


def _dedup_ldweights(nc):
    """Drop redundant InstLdweights from the compiled BIR.

    The rust lowering splits every matmul into InstLdweights +
    InstMatmult(ldweights=False), one load per matmul, even when
    consecutive matmuls use the identical stationary tile (the two
    512-col halves of each K-tile pair). The exposed load costs ~50ns
    per matmul on the PE array (measured 263ns/MM vs 213ns streaming).
    Here we remove an InstLdweights when (a) the previous InstLdweights
    in the same block loaded the exact same weights (same lowered AP,
    perf_mode, tile_position, transpose), (b) no other array-state
    change sits between them (only InstLdweights writes the array;
    matmuls/semaphores don't), and (c) it carries no semaphore waits or
    updates (so dropping it cannot break synchronization -- the waits
    that matter were moved onto the FIRST load of each pair by
    move_matmul_waits_to_ldweights).
    """
    removed = kept = 0
    for b in nc.m.functions[0].blocks:
        insts = b.instructions
        drop = []
        prev_key = None
        for i, ins in enumerate(insts):
            if isinstance(ins, mybir.InstLdweights):
                key = (
                    str(ins.ins[0]),
                    str(ins.perf_mode),
                    str(ins.tile_position),
                    str(ins.is_transpose),
                )
                si = ins.sync_info
                clean = si is None or (
                    len(si.on_wait) == 0 and len(si.on_update) == 0
                )
                if key == prev_key and clean:
                    drop.append(i)
                    removed += 1
                else:
                    kept += 1
                prev_key = key
        for i in reversed(drop):
            del insts[i]
    return removed, kept


def _build():
    nc = bacc.Bacc("TRN2", target_bir_lowering=False, debug=False, num_devices=NCORES)

    # All bulk inputs are partition-major on the host ([128, ktiles, free])
    # so DMAs are cheap-descriptor 2D patterns at full bandwidth.
    xt_d = nc.dram_tensor("xt", [128, KT_IN, BL], BF16, kind="ExternalInput")
    w1_d = nc.dram_tensor("w1t", [128, KT_IN, HID], FP8, kind="ExternalInput")
    w2_d = nc.dram_tensor("w2t", [128, JT, HID], FP8, kind="ExternalInput")
    w3_d = nc.dram_tensor("w3t", [128, JT, HID], FP8, kind="ExternalInput")
    w4_d = nc.dram_tensor("w4t", [128, JT, CLSP], BF16, kind="ExternalInput")
    bnp_d = nc.dram_tensor("bnp", [128, 6 * JT], F32, kind="ExternalInput")
    out_d = nc.dram_tensor("out", [CLSP, BL], F32, kind="ExternalOutput")

    nhalves = [(s, min(512, BL - s)) for s in range(0, BL, 512)]

    with tile.TileContext(nc) as tc:
        with (
            tc.tile_pool(name="weights", bufs=1) as wpool,
            tc.tile_pool(name="acts", bufs=1) as apool,
            tc.tile_pool(name="scratch", bufs=2) as scrpool,
            tc.tile_pool(name="stats", bufs=2) as spool,
            tc.tile_pool(name="psum", bufs=4, space="PSUM") as pspool,
            tc.tile_pool(name="dram", bufs=2, space="DRAM") as dpool,
        ):
            XT = wpool.tile([128, KT_IN, BL], BF16, tag="XT")
            W1 = wpool.tile([128, KT_IN, HID], FP8, tag="W1")
            W2 = wpool.tile([128, JT, HID], FP8, tag="W2")
            W3 = wpool.tile([128, JT, HID], FP8, tag="W3")
            W4 = wpool.tile([128, JT, CLSP], BF16, tag="W4")
            BNP = wpool.tile([128, 6 * JT], F32, tag="BNP")
            HRAW = apool.tile([128, JT, BL], BF16, tag="HRAW")
            H = apool.tile([128, JT, BL], BF16, tag="H")

            # One warmup collective: absorbs the ncfw wake + init cost
            # (~60-110us, run-variable) off the critical path. Input is an
            # unwritten scratch buffer (contents irrelevant); output
            # anchored into an unused out_d row at program end so DCE
            # keeps it. Collective latency stays elevated (~12-40us/mesh,
            # environmental) until ncfw's background init finishes around
            # 150us — extra or shape-matched warmups were measured NOT to
            # help, so keep exactly one and minimize early-era collectives.
            wu_outs = []
            for wi, cols in enumerate((1,)):
                win = dpool.tile([128, cols], F32, tag=f"wu_in{wi}")
                wo = dpool.tile([NCORES * 128, cols], F32, tag=f"wu_out{wi}")
                nc.gpsimd.collective_compute(
                    "AllGather",
                    mybir.AluOpType.bypass,
                    replica_groups=[list(range(NCORES))],
                    ins=[win.opt()],
                    outs=[wo.opt()],
                )
                wu_outs.append(wo)

            # Warm the Scalar activation tables (Relu + Rsqrt) at program
            # start so no ACT_TABLE_LOAD (~1.3us) lands mid-kernel on the
            # stats critical path. Input is uninitialized scratch (values
            # irrelevant, output discarded).
            WARM = scrpool.tile([128, 1], F32, tag="warm")
            WARMO = scrpool.tile([128, 1], F32, tag="warmo")
            nc.vector.memset(WARM[:], 1.0)
            nc.scalar.activation(WARMO[:], WARM[:], RELU)
            nc.scalar.activation(WARMO[:], WARM[:], SQRT)
            nc.scalar.activation(
                WARMO[:], WARM[:], mybir.ActivationFunctionType.Identity
            )

            # Input feed: XT on the Sync HWDGE ring, W1 on the Scalar HWDGE
            # ring, in progressively larger chunks so the first accumulation
            # chains start early while the bulk still moves in fat transfers.
            nc.sync.dma_start(BNP[:], bnp_d[:])
            # Tiny first transfers so the very first matmul (k=0, j=0,
            # batch half 0) can start ~1.5us earlier, then the bulk feed.
            nc.sync.dma_start(XT[:, 0:1, 0:512], xt_d[:, 0:1, 0:512])
            nc.scalar.dma_start(W1[:, 0:1, 0:128], w1_d[:, 0:1, 0:128])
            nc.sync.dma_start(XT[:, 0:1, 512:BL], xt_d[:, 0:1, 512:BL])
            nc.scalar.dma_start(W1[:, 0:1, 128:HID], w1_d[:, 0:1, 128:HID])
            feed = [1, 2, 2, 2, 2, 2, 4, 4, 4]
            c = 1
            for w in feed:
                w = min(w, KT_IN - c)
                if w <= 0:
                    break
                nc.sync.dma_start(XT[:, c : c + w, :], xt_d[:, c : c + w, :])
                nc.scalar.dma_start(W1[:, c : c + w, :], w1_d[:, c : c + w, :])
                c += w
            # Remaining weights behind the layer-1 feed on both rings.
            nc.sync.dma_start(W2[:], w2_d[:])
            nc.scalar.dma_start(W3[:], w3_d[:])
            nc.scalar.dma_start(W4[:], w4_d[:])

            def mm_pair(ps, Wk, rhs, k, kt):
                # The redundant second-half LDWEIGHTS is removed post-compile
                # by _dedup_ldweights (the in-IR ldweights flag is overwritten
                # by the rust split pass, so mutating it here does nothing).
                for idx, (s, w) in enumerate(nhalves):
                    nc.tensor.matmul(
                        ps[:, s : s + w],
                        Wk,
                        rhs[:, k, s : s + w],
                        start=(k == 0),
                        stop=(k == kt - 1),
                    )

            def relu_square(ps, jt, S, j, n):
                # relu: PSUM f32 -> SBUF bf16 on ScalarE; accum = batch sum.
                # sum of squares on VectorE from HRAW (hardware allows only
                # ONE psum operand per DVE op, so it can't read ps twice);
                # Vector is otherwise light, so all squares live there and
                # pipeline one tile behind the relus.
                nc.scalar.activation(
                    HRAW[:, jt, :], ps[:], RELU,
                    accum_out=S[:, j : j + 1],
                )
                scr = scrpool.tile([128, BL], BF16, tag="scr")
                nc.vector.scalar_tensor_tensor(
                    scr[:], HRAW[:, jt, :], 0.0, HRAW[:, jt, :],
                    mybir.AluOpType.bypass, MUL,
                    accum_out=S[:, n + j : n + j + 1],
                )

            def bn_ar_start(li, h, S, n):
                """Stats out + AllGather trigger for one feature group.

                Both ride the GpSimd queue, ordered so a group's stats
                export always issues before any earlier group's collective
                wait — exports are never stuck behind an unfinished
                collective. AllGather + local tree-sum beats AllReduce
                here: its mesh program is ~2x shorter on the serial CC
                engine, which is the scarce resource."""
                cc_in = dpool.tile([128, 2 * n], F32, tag="cc_in",
                                   name=f"cc_in_{li}_{h}")
                cc_out = dpool.tile([NCORES * 128, 2 * n], F32, tag="cc_out",
                                    name=f"cc_out_{li}_{h}")
                nc.gpsimd.dma_start(cc_in[:], S[:])
                nc.gpsimd.collective_compute(
                    "AllGather",
                    mybir.AluOpType.bypass,
                    replica_groups=[list(range(NCORES))],
                    ins=[cc_in.opt()],
                    outs=[cc_out.opt()],
                )
                return cc_out

            def bn_finish(li, h, jts, cc_out, Hdst, apply_out=True):
                """Pull gathered stats, compute a/c for `jts`, apply to Hdst.

                Split across queues so nothing upstream is ever gated by the
                collective wait: the gather-in rides the Sync ring (idle
                once the feed drains — on the gpsimd/CC ring it queues
                behind the NEXT collective's internal data movement and
                lands ~25us late); the core-sum reduce + stats math ride
                Vector; the one Sqrt rides Scalar between relu batches."""
                n = len(jts)
                # Gather [part, core, stat]: per-core contiguous runs keep
                # the DMA a fat 2D pattern. (A [part, stat, core] transpose
                # would allow a single innermost reduce, but lowers to an
                # element-strided DMA that takes ~17us — measured.)
                GAT = spool.tile([128, NCORES, 2 * n], F32, tag="GAT",
                                 name=f"GAT_{li}_{h}")
                nc.sync.dma_start(
                    GAT[:], cc_out.opt().rearrange("(c p) s -> p c s", p=128)
                )
                T4 = spool.tile([128, 4, 2 * n], F32, tag="T4", name=f"T4_{li}_{h}")
                nc.vector.tensor_tensor(T4[:], GAT[:, 0:4, :], GAT[:, 4:8, :], ADD)
                T2 = spool.tile([128, 2, 2 * n], F32, tag="T2", name=f"T2_{li}_{h}")
                nc.vector.tensor_tensor(T2[:], T4[:, 0:2, :], T4[:, 2:4, :], ADD)
                SS = spool.tile([128, 2 * n], F32, tag="SS", name=f"SS_{li}_{h}")
                nc.vector.tensor_tensor(SS[:], T2[:, 0, :], T2[:, 1, :], ADD)
                # a = g*rsqrt(var); c = beta - a*mean, computed scale-free:
                # V = E2 - E1^2/B = B*var (EPS dropped: shifts rsqrt ~5e-6
                # rel, far below the 2e-2 budget); host pre-scales gamma by
                # sqrt(B) so a = g' * sqrt(1/(B*var)) — the MEAN/MSQ ops of
                # the old chain are gone entirely.
                T = spool.tile([128, n], F32, tag="T", name=f"T_{li}_{h}")
                nc.vector.scalar_tensor_tensor(
                    T[:], SS[:, 0:n], 1.0 / B, SS[:, 0:n], MUL, MUL
                )
                V = spool.tile([128, n], F32, tag="V", name=f"V_{li}_{h}")
                nc.vector.tensor_tensor(V[:], SS[:, n : 2 * n], T[:], SUB)
                RI = spool.tile([128, n], F32, tag="RI", name=f"RI_{li}_{h}")
                nc.vector.reciprocal(RI[:], V[:])
                R = spool.tile([128, n], F32, tag="R", name=f"R_{li}_{h}")
                nc.scalar.activation(R[:], RI[:], SQRT)
                g0 = (2 * li) * JT + jts[0]
                b0 = (2 * li + 1) * JT + jts[0]
                A = spool.tile([128, n], F32, tag="A", name=f"A_{li}_{h}")
                nc.vector.tensor_tensor(A[:], R[:], BNP[:, g0 : g0 + n], MUL)
                AM = spool.tile([128, n], F32, tag="AM", name=f"AM_{li}_{h}")
                nc.vector.scalar_tensor_tensor(
                    AM[:], SS[:, 0:n], 1.0 / B, A[:], MUL, MUL
                )
                C = spool.tile([128, n], F32, tag="C", name=f"C_{li}_{h}")
                nc.vector.tensor_tensor(C[:], BNP[:, b0 : b0 + n], AM[:], SUB)
                if apply_out:
                    # All applies on Vector: tensor_scalar is ~0.46us/tile
                    # vs ~1.07us for a Scalar Identity activation, so
                    # engine-alternation is a net loss (measured).
                    for j, jt in enumerate(jts):
                        nc.vector.tensor_scalar(
                            Hdst[:, jt, :],
                            HRAW[:, jt, :],
                            A[:, j : j + 1],
                            C[:, j : j + 1],
                            MUL,
                            ADD,
                        )
                return A, C

            def mlp_layer(li, kt, rhs, W, Hdst, groups, apply_out=True):
                """One layer: matmuls + relu + distributed BN into Hdst.

                The first 4 feature tiles run k-outer (so layer-1 consumes
                input tiles in DMA-arrival order), the rest as sequential
                chains. `groups` partitions the 8 tiles into stats
                AllGathers: ncfw runs its first ~3 collectives slowly
                (12-27us) and later ones at ~5us, so layer 1 uses [6,2]
                (fewest slow-era ops) while layers 2-3 use [4,4] (group-0
                applies land before the layer ends; only one trailing
                mesh). Each finish(g) is issued right after ar_start(g):
                its Vector math sits before the NEXT group's squares and
                its Scalar sqrt between relu batches, so stats production
                is never queued behind a collective-dependent op, while
                applies land as early as dependencies allow."""
                done = 0
                acs = []
                for gi, n in enumerate(groups):
                    jts = list(range(done, done + n))
                    done += n
                    S = spool.tile([128, 2 * n], F32, tag=f"S_g{gi}",
                                   name=f"S{li}_{gi}")
                    for i, jt in enumerate(jts):
                        if jt == 0:
                            # tiles 0-3: k-outer over 4 concurrent chains
                            pss = [
                                pspool.tile([128, BL], F32, tag="ps",
                                            name=f"ps_g{j}")
                                for j in range(4)
                            ]
                            for k in range(kt):
                                for j in range(4):
                                    mm_pair(
                                        pss[j], W[:, k, j * 128 : (j + 1) * 128],
                                        rhs, k, kt,
                                    )
                        if jt < 4:
                            relu_square(pss[jt], jt, S, i, n)
                        else:
                            ps = pspool.tile([128, BL], F32, tag="ps",
                                             name=f"ps_s{jt}")
                            for k in range(kt):
                                mm_pair(
                                    ps, W[:, k, jt * 128 : (jt + 1) * 128],
                                    rhs, k, kt,
                                )
                            relu_square(ps, jt, S, i, n)
                    cc = bn_ar_start(li, gi, S, n)
                    acs.append(
                        (bn_finish(li, gi, jts, cc, Hdst, apply_out), jts)
                    )
                return acs

            # ---- layers ----
            H2 = apool.tile([128, JT, BL], BF16, tag="H2")
            # Group choice per layer: meshes cost ~22us END-TO-END FROM THEIR
            # START while ncfw background-init runs (first ~145us of the
            # kernel), ~5-7us after, and the CC engine runs them serially
            # (measured: an [8] single L1 mesh starting at 122 ended 144 —
            # no better than [6,2]'s second mesh, but it starved L2 of the
            # first 6 tiles). L1 [6,2]: g0 exports at ~91 (j0-5 done), mesh
            # ends ~114-121 delivering 6/8 k-tiles for L2's first phase;
            # g1 mesh ends ~140-146. L2 [4,2,2] / L3 [4,4]: warm-era meshes
            # pipeline behind the j4-7 chains; the k-outer j0-3 structure
            # consumes k ascending so the grouped-apply staircase never
            # head-of-line-blocks the PE queue.
            mlp_layer(0, KT_IN, XT, W1, H, groups=[6, 2])
            mlp_layer(1, JT, H, W2, H2, groups=[4, 4])
            # L3 [6,2]: the LAST group gates the kernel's end-chain (its mesh
            # + stats + the L4 k-tiles it unlocks) — keep it to 2 tiles so
            # the final chain is 4 main + 2 bias matmuls, while the [6]
            # group's mesh completes during L3's own j6/j7 chains and lets
            # L4 chew k0-5 (warm, behind the keep-warm dummies) early.
            l3 = mlp_layer(2, JT, H2, W3, H, groups=[6, 2], apply_out=False)

            # ---- layer 4 (no relu/BN) ----
            # BN3 is folded into L4's weights instead of applied to H:
            #   out_j = sum_f W4[j,f] (a_f r_f + c_f)
            #         = sum_f (W4[j,f] a_f) r_f  +  sum_f (W4[j,f] a_f)(c_f/a_f)
            # so per k-tile we scale W4 by a (per-partition, 16-wide — ~60ns)
            # and accumulate one extra 1-column matmul of c/a into a bias
            # psum, SHARING the scaled stationary weights (no extra
            # LDWEIGHTS after dedup). L4's matmuls start right after each
            # group's a/c lands — no serial H-apply chain on the tail.
            W4S = wpool.tile([128, JT, CLSP], BF16, tag="W4S")
            ps4 = pspool.tile([CLSP, BL], F32, tag="ps", name="ps4")
            psb = pspool.tile([CLSP, 1], F32, tag="ps", name="psb")
            # Keep-warm: the PE idles ~5-9us at each L3 stats-mesh wait and
            # HAM throttles it back to the 1.2GHz p-state (>3us idle), making
            # L4's matmuls run at ~427ns instead of ~263ns. Fill the two
            # known idle windows with dummy matmuls on resident data (same
            # stationary tile -> single deduped LDWEIGHTS; psum never read).
            def keep_warm(tag, pairs, gate_jt):
                # moving operand = a LATE L3 relu output, so the scheduler
                # cannot hoist these into earlier idle windows (measured:
                # with always-ready operands they get scheduled early and
                # the L4 matmuls still start cold).
                psw = pspool.tile([128, 512], F32, tag="ps", name=tag)
                for _ in range(pairs):
                    nc.tensor.matmul(
                        psw[:, 0:512],
                        W3[:, 0, 0:128],
                        HRAW[:, gate_jt, 0:512],
                        start=True,
                        stop=True,
                    )

            keep_warm("ps_kw0", 8, 5)
            for gi, ((A, C), jts) in enumerate(l3):
                if gi == 1:
                    keep_warm("ps_kw1", 12, 7)
                n = len(jts)
                AR = spool.tile([128, n], F32, tag="AR", name=f"AR4_{gi}")
                nc.vector.reciprocal(AR[:], A[:])
                CA = spool.tile([128, n], BF16, tag="CA", name=f"CA4_{gi}")
                nc.vector.tensor_tensor(CA[:], C[:], AR[:], MUL)
                for i, k in enumerate(jts):
                    nc.vector.tensor_scalar_mul(
                        W4S[:, k, :], W4[:, k, :], A[:, i : i + 1]
                    )
                    # bias first: psb finishes before the last main pair, so
                    # the KS copy overlaps the final matmuls.
                    nc.tensor.matmul(
                        psb[:, 0:1],
                        W4S[:, k, :],
                        CA[:, i : i + 1],
                        start=(k == 0),
                        stop=(k == JT - 1),
                    )
                    mm_pair(ps4, W4S[:, k, :], HRAW, k, JT)
            # Drain PSUM->SBUF in halves on two engines in parallel (each
            # fused with the +bias), then DMA each half on its own ring.
            KS = spool.tile([CLSP, 1], F32, tag="KS")
            nc.scalar.copy(KS[:], psb[:])
            OUTS = spool.tile([CLSP, BL], F32, tag="OUTS")
            nc.scalar.activation(
                OUTS[:, 0:512], ps4[:, 0:512],
                mybir.ActivationFunctionType.Identity, bias=KS[:, 0:1],
            )
            nc.vector.tensor_scalar_add(OUTS[:, 512:BL], ps4[:, 512:BL], KS[:, 0:1])
            nc.sync.dma_start(out_d[:, 0:512], OUTS[:, 0:512])
            nc.scalar.dma_start(out_d[:, 512:BL], OUTS[:, 512:BL])
            # Warmup-output anchors (keep the warmup collectives from DCE).
            # Issued last so the scheduler can't wedge their long warmup-
            # waits into the middle of the input feed; on the GpSimd ring
            # (idle after the last stats trigger) so they run well before
            # the output DMAs instead of queuing behind them on Sync.
            for wi, wo in enumerate(wu_outs):
                nc.gpsimd.dma_start(
                    out_d[CLSP - 1 : CLSP, wi : wi + 1], wo[0:1, 0:1]
                )

    nc.compile()
    _dedup_ldweights(nc)
    return nc


def _get_nc():
    if "nc" not in _CACHE:
        _CACHE["nc"] = _build()
    return _CACHE["nc"]


def _prep_inputs(x, W1, W2, W3, W4, g1, b1, g2, b2, g3, b3):
    x2 = np.asarray(x, dtype=np.float32).reshape(B, KIN)
    xt = np.ascontiguousarray(x2.T).astype(ml_dtypes.bfloat16)  # [3072, 8192]

    def pmajor(a):
        # [ktiles*128, free] -> [128, ktiles, free] (partition-major)
        kt = a.shape[0] // 128
        return np.ascontiguousarray(
            a.reshape(kt, 128, a.shape[1]).transpose(1, 0, 2)
        )

    def bin_t(w, pad=None, dtype=ml_dtypes.float8_e4m3):
        wb = np.where(np.asarray(w, dtype=np.float32) >= 0, 1.0, -1.0)
        wt = np.ascontiguousarray(wb.T).astype(dtype)  # [in, out], +-1 exact
        if pad is not None and wt.shape[1] < pad:
            wt = np.concatenate(
                [wt, np.zeros((wt.shape[0], pad - wt.shape[1]), wt.dtype)], axis=1
            )
        return pmajor(wt)

    w1t = bin_t(W1)            # [128, 24, 1024]
    w2t = bin_t(W2)            # [128, 8, 1024]
    w3t = bin_t(W3)
    # W4 ships bf16: it gets rescaled by the BN3 "a" on-device (W4' trick)
    w4t = bin_t(W4, pad=CLSP, dtype=ml_dtypes.bfloat16)  # [128, 8, 16]

    bnp = np.zeros((128, 6 * JT), dtype=np.float32)
    for l, p in enumerate([g1, b1, g2, b2, g3, b3]):
        pa = np.asarray(p, dtype=np.float32)
        if l % 2 == 0:
            # gammas pre-scaled by sqrt(B): the device computes
            # a = g' * rsqrt(B*var) without ever forming var itself.
            pa = pa * np.sqrt(float(B))
        for jt in range(JT):
            bnp[:, l * JT + jt] = pa[jt * 128 : (jt + 1) * 128]

    shared = {"w1t": w1t, "w2t": w2t, "w3t": w3t, "w4t": w4t, "bnp": bnp}
    in_maps = []
    for c in range(NCORES):
        m = dict(shared)
        m["xt"] = pmajor(np.ascontiguousarray(xt[:, c * BL : (c + 1) * BL]))
        in_maps.append(m)
    return in_maps


def _run(inputs, trace=False):
    nc = _get_nc()
    in_maps = _prep_inputs(**inputs)
    res = bass_utils.run_bass_kernel_spmd(
        nc, in_maps, core_ids=list(range(NCORES)), trace=trace
    )
    out = np.empty((B, CLS), dtype=np.float32)
    for c in range(NCORES):
        out[c * BL : (c + 1) * BL, :] = res.results[c]["out"][:CLS, :].T
    return out, res


def kernel(**inputs):
    out, _ = _run(inputs, trace=False)
    return out

